# revision 2
# baseline (speedup 1.0000x reference)
"""AveragePrecision kernel v3 for Trainium2, 8 NeuronCores.

Histogram via 4-amplitude-packed one-hot matmul (psum [128,128] fp32):
  per point k: amp_k * [t_k mod 128 == m] * [i_k mod 128 == n],
  amp_k = 4096^(t_k>=128) * 64^(i_k>=128) in {1,64,4096,262144}.
  Six-bit fields, exact in fp32 while every per-core field count < 64
  (expected ~16 for the graded input; host-side validity check with
  numpy fallback).

Builder mix per 64-chunk superblock (measured steady-state costs):
  - lhs (stationary, t-one-hot * amp):
      batches 0-6: GPSIMD batched local_scatter, 8 chunks/call (~138 ns/chunk)
      batch 7:     DVE tensor_scalar(is_equal, mult) per chunk (~228 ns)
  - rhs (moving, plain i-one-hot):
      26 of 32 pairs: one batched TT is_equal per 2 chunks against an
        interleaved iota (iota_int2[p,2n+j]=n, in1 minor-dim [1,2] keeps
        the DVE 2x perf mode, ~104 ns/chunk); the matmul consumes the
        pair tile through a stride-2 AP (4-byte step = full PE speed).
      6 of 32 pairs: ACT square+relu one-hot, contiguous (~659 ns/chunk)
  - PE: one ldweights+matmul(N=128) per chunk (~65 ns).
"""

import sys
import types

sys.path.insert(0, "/opt/trn_rl_repo")

if "antenv.axon_hooks" not in sys.modules:
    _hooks = types.ModuleType("antenv.axon_hooks")
    _hooks._HOOK = None

    def _get_hook():
        if _hooks._HOOK is None:
            try:
                from trn_agent_boot.trn_boot import _ntff_profile_via_ctypes

                _hooks._HOOK = _ntff_profile_via_ctypes("/opt/axon/libaxon_pjrt.so")
            except Exception:
                _hooks._HOOK = None
        return _hooks._HOOK

    def _set_hook(h):
        _hooks._HOOK = h

    _hooks.get_axon_ntff_profile_hook = _get_hook
    _hooks.set_axon_ntff_profile_hook = _set_hook
    sys.modules["antenv.axon_hooks"] = _hooks

import numpy as np

N_TOTAL = 8_388_608
C = 256
IOU_TH = 0.5
NCORES = 8
N_PER_CORE = N_TOTAL // NCORES
P = 128
W = N_PER_CORE // P                     # 8192 chunks per core

_compiled = {}

SB = 8            # gpsimd local_scatter batch (chunks per call)
SUPER = 64        # superblock: 8 batches
DVE_LHS_BATCHES = {7}  # batches (of SUPER//SB) built by DVE instead of GPSIMD
ACT_PAIRS = {2, 7, 12, 17, 22, 27}  # pairs (of SUPER//2) built by ACT


def _build_program_v3(w=W):
    import concourse.bass as bass
    import concourse.mybir as mybir
    import concourse.tile as tile
    from concourse import bacc

    nc = bacc.Bacc("TRN2", target_bir_lowering=False, debug=False, num_devices=NCORES)

    inp = nc.dram_tensor("inp", [P, w], mybir.dt.int32, kind="ExternalInput").ap()
    tgt = nc.dram_tensor("tgt", [P, w], mybir.dt.int32, kind="ExternalInput").ap()
    hist = nc.dram_tensor("hist", [P, 128], mybir.dt.float32, kind="ExternalOutput").ap()

    BF16 = mybir.dt.bfloat16
    FP32 = mybir.dt.float32
    I16 = mybir.dt.int16
    I32 = mybir.dt.int32
    EQ = mybir.AluOpType.is_equal
    GE = mybir.AluOpType.is_ge
    MULT = mybir.AluOpType.mult
    ADD = mybir.AluOpType.add
    AF = mybir.ActivationFunctionType

    W_IN = 512

    with tile.TileContext(nc) as tc:
        with (
            tc.tile_pool(name="persist", bufs=1) as persist,
            tc.tile_pool(name="stage", bufs=2) as stage,
            tc.tile_pool(name="lhsp", bufs=6) as lhspool,
            tc.tile_pool(name="lhsd", bufs=12) as lhsdpool,
            tc.tile_pool(name="rhsp", bufs=16) as rhspool,
            tc.tile_pool(name="psum", bufs=1, space="PSUM") as psum_pool,
        ):
            iota128 = persist.tile([P, 128], I16, tag="iota128")
            nc.gpsimd.iota(iota128[:, :], pattern=[[1, 128]], base=0, channel_multiplier=0)
            # interleaved iota: iota_int2[p, 2n+j] = n
            iota_int2 = persist.tile([P, 256], I16, tag="iota_int2")
            nc.gpsimd.iota(iota_int2[:, :], pattern=[[1, 128], [0, 2]], base=0, channel_multiplier=0)
            # scatter offsets 128*(c mod SB), periodic; W_IN % SB == 0
            off16 = persist.tile([P, W_IN], I16, tag="off16")
            nc.gpsimd.iota(off16[:, :], pattern=[[0, W_IN // SB], [128, SB]], base=0, channel_multiplier=0)

            imodf = persist.tile([P, w], FP32, tag="imodf")    # ACT bias (fp32)
            imod16 = persist.tile([P, w], I16, tag="imod16")   # TT compare (int16)
            tmodf = persist.tile([P, w], FP32, tag="tmodf")    # DVE lhs scalar1
            ampf = persist.tile([P, w], FP32, tag="ampf")      # DVE lhs scalar2
            idx_flat = persist.tile([P, w], I16, tag="idxf")   # tm + 128*(c%SB)
            amp_flat = persist.tile([P, w], BF16, tag="ampb")  # scatter data

            for s in range(0, w, W_IN):
                ws = min(W_IN, w - s)
                st = stage.tile([P, W_IN], I32, tag="st_t")
                nc.sync.dma_start(out=st[:, :ws], in_=tgt[:, s : s + ws])
                si = stage.tile([P, W_IN], I32, tag="st_i")
                nc.sync.dma_start(out=si[:, :ws], in_=inp[:, s : s + ws])

                t7 = stage.tile([P, W_IN], FP32, tag="t7")
                nc.vector.tensor_scalar(out=t7[:, :ws], in0=st[:, :ws], scalar1=127.5, scalar2=None, op0=GE)
                i7 = stage.tile([P, W_IN], FP32, tag="i7")
                nc.vector.tensor_scalar(out=i7[:, :ws], in0=si[:, :ws], scalar1=127.5, scalar2=None, op0=GE)

                nc.vector.scalar_tensor_tensor(out=imodf[:, s : s + ws], in0=i7[:, :ws], scalar=-128.0, in1=si[:, :ws], op0=MULT, op1=ADD)
                nc.vector.scalar_tensor_tensor(out=tmodf[:, s : s + ws], in0=t7[:, :ws], scalar=-128.0, in1=st[:, :ws], op0=MULT, op1=ADD)
                nc.vector.tensor_copy(out=imod16[:, s : s + ws], in_=imodf[:, s : s + ws])

                # amp = (1 + 4095*t7) * (1 + 63*i7); in-place on t7/i7
                nc.vector.tensor_scalar(out=t7[:, :ws], in0=t7[:, :ws], scalar1=4095.0, scalar2=1.0, op0=MULT, op1=ADD)
                nc.vector.tensor_scalar(out=i7[:, :ws], in0=i7[:, :ws], scalar1=63.0, scalar2=1.0, op0=MULT, op1=ADD)
                nc.vector.tensor_tensor(out=ampf[:, s : s + ws], in0=t7[:, :ws], in1=i7[:, :ws], op=MULT)
                nc.vector.tensor_copy(out=amp_flat[:, s : s + ws], in_=ampf[:, s : s + ws])

                # idx = tmod + 128*(c mod SB)
                tm16 = stage.tile([P, W_IN], I16, tag="tm16")
                nc.vector.tensor_copy(out=tm16[:, :ws], in_=tmodf[:, s : s + ws])
                nc.vector.tensor_tensor(out=idx_flat[:, s : s + ws], in0=tm16[:, :ws], in1=off16[:, :ws], op=ADD)

            psum = psum_pool.tile([P, 128], FP32, tag="psum")

            n_batches = w // SB
            for b in range(n_batches):
                c0 = b * SB
                in_super = b % (SUPER // SB)
                if in_super not in DVE_LHS_BATCHES:
                    lhsb = lhspool.tile([P, SB * 128], BF16, tag="lhsb")
                    nc.gpsimd.local_scatter(
                        out_ap=lhsb[:, :],
                        data_ap=amp_flat[:, c0 : c0 + SB],
                        idxs_ap=idx_flat[:, c0 : c0 + SB],
                        channels=P, num_elems=SB * 128, num_idxs=SB,
                    )
                    lhs_of = lambda j, _t=lhsb: _t[:, 128 * j : 128 * (j + 1)]
                else:
                    dve_tiles = []
                    for j in range(SB):
                        c = c0 + j
                        lt = lhsdpool.tile([P, 128], BF16, tag="lhsd")
                        nc.vector.tensor_scalar(
                            out=lt[:, :], in0=iota128[:, :],
                            scalar1=tmodf[:, c : c + 1], scalar2=ampf[:, c : c + 1],
                            op0=EQ, op1=MULT,
                        )
                        dve_tiles.append(lt)
                    lhs_of = lambda j, _l=dve_tiles: _l[j][:, :]

                # rhs pairs within this batch: chunks (c0+2q, c0+2q+1)
                for q in range(SB // 2):
                    c = c0 + 2 * q
                    pair_idx = (c % SUPER) // 2
                    if pair_idx in ACT_PAIRS:
                        rhs_list = []
                        for j in range(2):
                            sq = rhspool.tile([P, 128], FP32, tag="sq")
                            nc.scalar.activation(
                                out=sq[:, :], in_=iota128[:, :], func=AF.Square,
                                bias=imodf[:, c + j : c + j + 1], scale=-1.0,
                            )
                            rt = rhspool.tile([P, 128], BF16, tag="rhs_a")
                            nc.scalar.activation(
                                out=rt[:, :], in_=sq[:, :], func=AF.Relu,
                                bias=1.0, scale=-1.0,
                            )
                            rhs_list.append(rt)
                        rhs_aps = [rhs_list[0][:, :], rhs_list[1][:, :]]
                    else:
                        prt = rhspool.tile([P, 256], BF16, tag="rhs_p")
                        in1 = bass.AP(imod16.tensor, c, [[w, P], [0, 128], [1, 2]])
                        nc.vector.tensor_tensor(out=prt[:, :], in0=iota_int2[:, :], in1=in1, op=EQ)
                        rhs_aps = [
                            bass.AP(prt.tensor, 0, [[256, P], [2, 128]]),
                            bass.AP(prt.tensor, 1, [[256, P], [2, 128]]),
                        ]
                    for j in range(2):
                        cc = c + j
                        nc.tensor.matmul(
                            psum[:, :], lhs_of(2 * q + j), rhs_aps[j],
                            start=(cc == 0), stop=(cc == w - 1),
                        )

            out_sb = persist.tile([P, 128], FP32, tag="out_sb")
            nc.vector.tensor_copy(out=out_sb[:, :], in_=psum[:, :])
            nc.sync.dma_start(out=hist[:, :], in_=out_sb[:, :])

    nc.compile()
    return nc


def _get_program(w=W, kind="v3"):
    key = (kind, w)
    if key not in _compiled:
        _compiled[key] = _build_program_v3(w)
    return _compiled[key]


def _decode_fields(h):
    v = np.rint(h).astype(np.int64)
    c00 = v & 63
    c01 = (v >> 6) & 63
    c10 = (v >> 12) & 63
    c11 = v >> 18
    inter = np.zeros((C, C), dtype=np.int64)
    inter[0:128, 0:128] = c00
    inter[0:128, 128:256] = c01
    inter[128:256, 0:128] = c10
    inter[128:256, 128:256] = c11
    return inter, (c00, c01, c10, c11)


def _histogram_device(input_np, target_np, w=W, trace=False):
    from concourse.bass_utils import run_bass_kernel_spmd

    n = NCORES * P * w
    inp = np.ascontiguousarray(input_np[:n].reshape(NCORES, P, w).astype(np.int32))
    tgt = np.ascontiguousarray(target_np[:n].reshape(NCORES, P, w).astype(np.int32))
    in_maps = [{"inp": inp[c], "tgt": tgt[c]} for c in range(NCORES)]

    nc = _get_program(w, "v3")
    try:
        res = run_bass_kernel_spmd(nc, in_maps, core_ids=list(range(NCORES)), trace=trace)
    except Exception:
        res = run_bass_kernel_spmd(nc, in_maps, core_ids=list(range(NCORES)), trace=trace)

    inter = np.zeros((C, C), dtype=np.int64)
    ok = True
    for cid in range(NCORES):
        h = res.results[cid]["hist"].astype(np.float64)
        ci, fields = _decode_fields(h)
        inter += ci
        if any(f.max() >= 63 for f in fields) or ci.sum() != P * w:
            ok = False
    if not ok:
        inter = np.zeros((C, C), dtype=np.int64)
        np.add.at(inter, (target_np[:n].reshape(-1), input_np[:n].reshape(-1)), 1)
        return inter.astype(np.float64), res
    return inter.astype(np.float64), res


def _finalize(inter64):
    inter = inter64.astype(np.float32)
    cnt_gt = inter.sum(axis=1, dtype=np.float32)
    cnt_pr = inter.sum(axis=0, dtype=np.float32)
    union = cnt_gt[:, None] + cnt_pr[None, :] - inter
    with np.errstate(divide="ignore", invalid="ignore"):
        iou = np.where(union > 0, inter / np.maximum(union, np.float32(1.0)), np.float32(0.0)).astype(np.float32)
    TP = (iou >= np.float32(IOU_TH)).astype(np.float32).sum(axis=1)
    FP = ((iou > 0) & (iou < np.float32(IOU_TH))).astype(np.float32).sum(axis=1)
    present = cnt_gt > 0
    precision = np.where(present, TP / np.maximum(TP + FP, np.float32(1.0)), np.float32(0.0)).astype(np.float32)
    n_gt = max(np.float32(present.astype(np.float32).sum()), np.float32(1.0))
    return np.float32(precision.sum(dtype=np.float32) / n_gt)


def kernel(input, target):
    input = np.asarray(input)
    target = np.asarray(target)
    inter, _ = _histogram_device(input, target)
    return np.array(_finalize(inter), dtype=np.float32)


if __name__ == "__main__":
    rng = np.random.default_rng(0)
    inp = rng.integers(0, C, size=N_TOTAL, dtype=np.int32)
    tgt = rng.integers(0, C, size=N_TOTAL, dtype=np.int32)
    out = kernel(input=inp, target=tgt)
    print("kernel output:", out)


# revision 4
# speedup vs baseline: 1.2052x; 1.2052x over previous
"""AveragePrecision kernel v3 for Trainium2, 8 NeuronCores.

Histogram via 4-amplitude-packed one-hot matmul (psum [128,128] fp32):
  per point k: amp_k * [t_k mod 128 == m] * [i_k mod 128 == n],
  amp_k = 4096^(t_k>=128) * 64^(i_k>=128) in {1,64,4096,262144}.
  Six-bit fields, exact in fp32 while every per-core field count < 64
  (expected ~16 for the graded input; host-side validity check with
  numpy fallback).

Builder mix per 64-chunk superblock (measured steady-state costs):
  - lhs (stationary, t-one-hot * amp):
      batches 0-6: GPSIMD batched local_scatter, 8 chunks/call (~138 ns/chunk)
      batch 7:     DVE tensor_scalar(is_equal, mult) per chunk (~228 ns)
  - rhs (moving, plain i-one-hot):
      26 of 32 pairs: one batched TT is_equal per 2 chunks against an
        interleaved iota (iota_int2[p,2n+j]=n, in1 minor-dim [1,2] keeps
        the DVE 2x perf mode, ~104 ns/chunk); the matmul consumes the
        pair tile through a stride-2 AP (4-byte step = full PE speed).
      6 of 32 pairs: ACT square+relu one-hot, contiguous (~659 ns/chunk)
  - PE: one ldweights+matmul(N=128) per chunk (~65 ns).
"""

import sys
import types

sys.path.insert(0, "/opt/trn_rl_repo")

if "antenv.axon_hooks" not in sys.modules:
    _hooks = types.ModuleType("antenv.axon_hooks")
    _hooks._HOOK = None

    def _get_hook():
        if _hooks._HOOK is None:
            try:
                from trn_agent_boot.trn_boot import _ntff_profile_via_ctypes

                _hooks._HOOK = _ntff_profile_via_ctypes("/opt/axon/libaxon_pjrt.so")
            except Exception:
                _hooks._HOOK = None
        return _hooks._HOOK

    def _set_hook(h):
        _hooks._HOOK = h

    _hooks.get_axon_ntff_profile_hook = _get_hook
    _hooks.set_axon_ntff_profile_hook = _set_hook
    sys.modules["antenv.axon_hooks"] = _hooks

import numpy as np

N_TOTAL = 8_388_608
C = 256
IOU_TH = 0.5
NCORES = 8
N_PER_CORE = N_TOTAL // NCORES
P = 128
W = N_PER_CORE // P                     # 8192 chunks per core

_compiled = {}

SB = 8            # gpsimd local_scatter batch (chunks per call)
SUPER = 64        # superblock: 8 batches
DVE_LHS_BATCHES = {7}  # batches (of SUPER//SB) built by DVE instead of GPSIMD
ACT_PAIRS = {2, 7, 12, 17, 22, 27}  # pairs (of SUPER//2) built by ACT
POOL_LHS = 6
POOL_LHSD = 12
POOL_RHS = 16


def _build_program_v3(w=W):
    import concourse.bass as bass
    import concourse.mybir as mybir
    import concourse.tile as tile
    from concourse import bacc

    nc = bacc.Bacc("TRN2", target_bir_lowering=False, debug=False, num_devices=NCORES)

    inp = nc.dram_tensor("inp", [P, w], mybir.dt.int32, kind="ExternalInput").ap()
    tgt = nc.dram_tensor("tgt", [P, w], mybir.dt.int32, kind="ExternalInput").ap()
    hist = nc.dram_tensor("hist", [P, 128], mybir.dt.float32, kind="ExternalOutput").ap()

    BF16 = mybir.dt.bfloat16
    FP32 = mybir.dt.float32
    I16 = mybir.dt.int16
    I32 = mybir.dt.int32
    EQ = mybir.AluOpType.is_equal
    GE = mybir.AluOpType.is_ge
    MULT = mybir.AluOpType.mult
    ADD = mybir.AluOpType.add
    AF = mybir.ActivationFunctionType

    W_IN = 512

    # trace_sim=True makes the tile scheduler take a deterministic path that
    # reliably lands the fast (~1.18 ms) schedule; without it, scheduling is
    # nondeterministic across processes (measured 1.17-1.39 ms spread).
    with tile.TileContext(nc, trace_sim=True) as tc:
        with (
            tc.tile_pool(name="persist", bufs=1) as persist,
            tc.tile_pool(name="stage", bufs=2) as stage,
            tc.tile_pool(name="lhsp", bufs=POOL_LHS) as lhspool,
            tc.tile_pool(name="lhsd", bufs=POOL_LHSD) as lhsdpool,
            tc.tile_pool(name="rhsp", bufs=POOL_RHS) as rhspool,
            tc.tile_pool(name="psum", bufs=1, space="PSUM") as psum_pool,
        ):
            iota128 = persist.tile([P, 128], I16, tag="iota128")
            nc.gpsimd.iota(iota128[:, :], pattern=[[1, 128]], base=0, channel_multiplier=0)
            # interleaved iota: iota_int2[p, 2n+j] = n
            iota_int2 = persist.tile([P, 256], I16, tag="iota_int2")
            nc.gpsimd.iota(iota_int2[:, :], pattern=[[1, 128], [0, 2]], base=0, channel_multiplier=0)
            # scatter offsets 128*(c mod SB), periodic; W_IN % SB == 0
            off16 = persist.tile([P, W_IN], I16, tag="off16")
            nc.gpsimd.iota(off16[:, :], pattern=[[0, W_IN // SB], [128, SB]], base=0, channel_multiplier=0)

            imodf = persist.tile([P, w], FP32, tag="imodf")    # ACT bias (fp32)
            imod16 = persist.tile([P, w], I16, tag="imod16")   # TT compare (int16)
            tmodf = persist.tile([P, w], FP32, tag="tmodf")    # DVE lhs scalar1
            ampf = persist.tile([P, w], FP32, tag="ampf")      # DVE lhs scalar2
            idx_flat = persist.tile([P, w], I16, tag="idxf")   # tm + 128*(c%SB)
            amp_flat = persist.tile([P, w], BF16, tag="ampb")  # scatter data

            for s in range(0, w, W_IN):
                ws = min(W_IN, w - s)
                st = stage.tile([P, W_IN], I32, tag="st_t")
                nc.sync.dma_start(out=st[:, :ws], in_=tgt[:, s : s + ws])
                si = stage.tile([P, W_IN], I32, tag="st_i")
                nc.sync.dma_start(out=si[:, :ws], in_=inp[:, s : s + ws])

                t7 = stage.tile([P, W_IN], FP32, tag="t7")
                nc.vector.tensor_scalar(out=t7[:, :ws], in0=st[:, :ws], scalar1=127.5, scalar2=None, op0=GE)
                i7 = stage.tile([P, W_IN], FP32, tag="i7")
                nc.vector.tensor_scalar(out=i7[:, :ws], in0=si[:, :ws], scalar1=127.5, scalar2=None, op0=GE)

                nc.vector.scalar_tensor_tensor(out=imodf[:, s : s + ws], in0=i7[:, :ws], scalar=-128.0, in1=si[:, :ws], op0=MULT, op1=ADD)
                nc.vector.scalar_tensor_tensor(out=tmodf[:, s : s + ws], in0=t7[:, :ws], scalar=-128.0, in1=st[:, :ws], op0=MULT, op1=ADD)
                nc.vector.tensor_copy(out=imod16[:, s : s + ws], in_=imodf[:, s : s + ws])

                # amp = (1 + 4095*t7) * (1 + 63*i7); in-place on t7/i7
                nc.vector.tensor_scalar(out=t7[:, :ws], in0=t7[:, :ws], scalar1=4095.0, scalar2=1.0, op0=MULT, op1=ADD)
                nc.vector.tensor_scalar(out=i7[:, :ws], in0=i7[:, :ws], scalar1=63.0, scalar2=1.0, op0=MULT, op1=ADD)
                nc.vector.tensor_tensor(out=ampf[:, s : s + ws], in0=t7[:, :ws], in1=i7[:, :ws], op=MULT)
                nc.vector.tensor_copy(out=amp_flat[:, s : s + ws], in_=ampf[:, s : s + ws])

                # idx = tmod + 128*(c mod SB)
                tm16 = stage.tile([P, W_IN], I16, tag="tm16")
                nc.vector.tensor_copy(out=tm16[:, :ws], in_=tmodf[:, s : s + ws])
                nc.vector.tensor_tensor(out=idx_flat[:, s : s + ws], in0=tm16[:, :ws], in1=off16[:, :ws], op=ADD)

            psum = psum_pool.tile([P, 128], FP32, tag="psum")

            n_batches = w // SB
            for b in range(n_batches):
                c0 = b * SB
                in_super = b % (SUPER // SB)
                if in_super not in DVE_LHS_BATCHES:
                    lhsb = lhspool.tile([P, SB * 128], BF16, tag="lhsb")
                    nc.gpsimd.local_scatter(
                        out_ap=lhsb[:, :],
                        data_ap=amp_flat[:, c0 : c0 + SB],
                        idxs_ap=idx_flat[:, c0 : c0 + SB],
                        channels=P, num_elems=SB * 128, num_idxs=SB,
                    )
                    lhs_of = lambda j, _t=lhsb: _t[:, 128 * j : 128 * (j + 1)]
                else:
                    dve_tiles = []
                    for j in range(SB):
                        c = c0 + j
                        lt = lhsdpool.tile([P, 128], BF16, tag="lhsd")
                        nc.vector.tensor_scalar(
                            out=lt[:, :], in0=iota128[:, :],
                            scalar1=tmodf[:, c : c + 1], scalar2=ampf[:, c : c + 1],
                            op0=EQ, op1=MULT,
                        )
                        dve_tiles.append(lt)
                    lhs_of = lambda j, _l=dve_tiles: _l[j][:, :]

                # rhs pairs within this batch: chunks (c0+2q, c0+2q+1)
                for q in range(SB // 2):
                    c = c0 + 2 * q
                    pair_idx = (c % SUPER) // 2
                    if pair_idx in ACT_PAIRS:
                        rhs_list = []
                        for j in range(2):
                            sq = rhspool.tile([P, 128], FP32, tag="sq")
                            nc.scalar.activation(
                                out=sq[:, :], in_=iota128[:, :], func=AF.Square,
                                bias=imodf[:, c + j : c + j + 1], scale=-1.0,
                            )
                            rt = rhspool.tile([P, 128], BF16, tag="rhs_a")
                            nc.scalar.activation(
                                out=rt[:, :], in_=sq[:, :], func=AF.Relu,
                                bias=1.0, scale=-1.0,
                            )
                            rhs_list.append(rt)
                        rhs_aps = [rhs_list[0][:, :], rhs_list[1][:, :]]
                    else:
                        prt = rhspool.tile([P, 256], BF16, tag="rhs_p")
                        in1 = bass.AP(imod16.tensor, c, [[w, P], [0, 128], [1, 2]])
                        nc.vector.tensor_tensor(out=prt[:, :], in0=iota_int2[:, :], in1=in1, op=EQ)
                        rhs_aps = [
                            bass.AP(prt.tensor, 0, [[256, P], [2, 128]]),
                            bass.AP(prt.tensor, 1, [[256, P], [2, 128]]),
                        ]
                    for j in range(2):
                        cc = c + j
                        nc.tensor.matmul(
                            psum[:, :], lhs_of(2 * q + j), rhs_aps[j],
                            start=(cc == 0), stop=(cc == w - 1),
                        )

            out_sb = persist.tile([P, 128], FP32, tag="out_sb")
            nc.vector.tensor_copy(out=out_sb[:, :], in_=psum[:, :])
            nc.sync.dma_start(out=hist[:, :], in_=out_sb[:, :])

    nc.compile()
    return nc


def _get_program(w=W, kind="v3"):
    key = (kind, w)
    if key not in _compiled:
        _compiled[key] = _build_program_v3(w)
    return _compiled[key]


def _decode_fields(h):
    v = np.rint(h).astype(np.int64)
    c00 = v & 63
    c01 = (v >> 6) & 63
    c10 = (v >> 12) & 63
    c11 = v >> 18
    inter = np.zeros((C, C), dtype=np.int64)
    inter[0:128, 0:128] = c00
    inter[0:128, 128:256] = c01
    inter[128:256, 0:128] = c10
    inter[128:256, 128:256] = c11
    return inter, (c00, c01, c10, c11)


def _histogram_device(input_np, target_np, w=W, trace=False):
    from concourse.bass_utils import run_bass_kernel_spmd

    n = NCORES * P * w
    inp = np.ascontiguousarray(input_np[:n].reshape(NCORES, P, w).astype(np.int32))
    tgt = np.ascontiguousarray(target_np[:n].reshape(NCORES, P, w).astype(np.int32))
    in_maps = [{"inp": inp[c], "tgt": tgt[c]} for c in range(NCORES)]

    nc = _get_program(w, "v3")
    try:
        res = run_bass_kernel_spmd(nc, in_maps, core_ids=list(range(NCORES)), trace=trace)
    except Exception:
        res = run_bass_kernel_spmd(nc, in_maps, core_ids=list(range(NCORES)), trace=trace)

    inter = np.zeros((C, C), dtype=np.int64)
    ok = True
    for cid in range(NCORES):
        h = res.results[cid]["hist"].astype(np.float64)
        ci, fields = _decode_fields(h)
        inter += ci
        if any(f.max() >= 63 for f in fields) or ci.sum() != P * w:
            ok = False
    if not ok:
        inter = np.zeros((C, C), dtype=np.int64)
        np.add.at(inter, (target_np[:n].reshape(-1), input_np[:n].reshape(-1)), 1)
        return inter.astype(np.float64), res
    return inter.astype(np.float64), res


def _finalize(inter64):
    inter = inter64.astype(np.float32)
    cnt_gt = inter.sum(axis=1, dtype=np.float32)
    cnt_pr = inter.sum(axis=0, dtype=np.float32)
    union = cnt_gt[:, None] + cnt_pr[None, :] - inter
    with np.errstate(divide="ignore", invalid="ignore"):
        iou = np.where(union > 0, inter / np.maximum(union, np.float32(1.0)), np.float32(0.0)).astype(np.float32)
    TP = (iou >= np.float32(IOU_TH)).astype(np.float32).sum(axis=1)
    FP = ((iou > 0) & (iou < np.float32(IOU_TH))).astype(np.float32).sum(axis=1)
    present = cnt_gt > 0
    precision = np.where(present, TP / np.maximum(TP + FP, np.float32(1.0)), np.float32(0.0)).astype(np.float32)
    n_gt = max(np.float32(present.astype(np.float32).sum()), np.float32(1.0))
    return np.float32(precision.sum(dtype=np.float32) / n_gt)


def kernel(input, target):
    input = np.asarray(input)
    target = np.asarray(target)
    inter, _ = _histogram_device(input, target)
    return np.array(_finalize(inter), dtype=np.float32)


if __name__ == "__main__":
    rng = np.random.default_rng(0)
    inp = rng.integers(0, C, size=N_TOTAL, dtype=np.int32)
    tgt = rng.integers(0, C, size=N_TOTAL, dtype=np.int32)
    out = kernel(input=inp, target=tgt)
    print("kernel output:", out)


# revision 5
# speedup vs baseline: 1.2497x; 1.0369x over previous
"""AveragePrecision kernel v3 for Trainium2, 8 NeuronCores.

Histogram via 4-amplitude-packed one-hot matmul (psum [128,128] fp32):
  per point k: amp_k * [t_k mod 128 == m] * [i_k mod 128 == n],
  amp_k = 4096^(t_k>=128) * 64^(i_k>=128) in {1,64,4096,262144}.
  Six-bit fields, exact in fp32 while every per-core field count < 64
  (expected ~16 for the graded input; host-side validity check with
  numpy fallback).

Builder mix per 64-chunk superblock (measured steady-state costs):
  - lhs (stationary, t-one-hot * amp):
      batches 0-6: GPSIMD batched local_scatter, 8 chunks/call (~138 ns/chunk)
      batch 7:     DVE tensor_scalar(is_equal, mult) per chunk (~228 ns)
  - rhs (moving, plain i-one-hot):
      26 of 32 pairs: one batched TT is_equal per 2 chunks against an
        interleaved iota (iota_int2[p,2n+j]=n, in1 minor-dim [1,2] keeps
        the DVE 2x perf mode, ~104 ns/chunk); the matmul consumes the
        pair tile through a stride-2 AP (4-byte step = full PE speed).
      6 of 32 pairs: ACT square+relu one-hot, contiguous (~659 ns/chunk)
  - PE: one ldweights+matmul(N=128) per chunk (~65 ns).
"""

import sys
import types

sys.path.insert(0, "/opt/trn_rl_repo")

if "antenv.axon_hooks" not in sys.modules:
    _hooks = types.ModuleType("antenv.axon_hooks")
    _hooks._HOOK = None

    def _get_hook():
        if _hooks._HOOK is None:
            try:
                from trn_agent_boot.trn_boot import _ntff_profile_via_ctypes

                _hooks._HOOK = _ntff_profile_via_ctypes("/opt/axon/libaxon_pjrt.so")
            except Exception:
                _hooks._HOOK = None
        return _hooks._HOOK

    def _set_hook(h):
        _hooks._HOOK = h

    _hooks.get_axon_ntff_profile_hook = _get_hook
    _hooks.set_axon_ntff_profile_hook = _set_hook
    sys.modules["antenv.axon_hooks"] = _hooks

import numpy as np

N_TOTAL = 8_388_608
C = 256
IOU_TH = 0.5
NCORES = 8
N_PER_CORE = N_TOTAL // NCORES
P = 128
W = N_PER_CORE // P                     # 8192 chunks per core

_compiled = {}

SB = 8            # gpsimd local_scatter batch (chunks per call)
SUPER = 64        # superblock: 8 batches
DVE_LHS_BATCHES = {3, 7}  # batches (of SUPER//SB) built by DVE instead of GPSIMD
ACT_PAIRS = {2, 7, 12, 17, 22, 27}  # pairs (of SUPER//2) built by ACT
SPLIT_BATCHES = set()  # batches half-split: B=4 scatter + 4 DVE chunks
RHS_QUAD = True  # rhs via B=4 interleaved TT (stride-4 matmul) instead of pairs
ACT_QUADS = {2, 7, 12}  # quad indices (of 16 per superblock) built by ACT when RHS_QUAD
POOL_LHS = 6
POOL_LHSD = 12
POOL_RHS = 10


def _build_program_v3(w=W):
    import concourse.bass as bass
    import concourse.mybir as mybir
    import concourse.tile as tile
    from concourse import bacc

    nc = bacc.Bacc("TRN2", target_bir_lowering=False, debug=False, num_devices=NCORES)

    inp = nc.dram_tensor("inp", [P, w], mybir.dt.int32, kind="ExternalInput").ap()
    tgt = nc.dram_tensor("tgt", [P, w], mybir.dt.int32, kind="ExternalInput").ap()
    hist = nc.dram_tensor("hist", [P, 128], mybir.dt.float32, kind="ExternalOutput").ap()

    BF16 = mybir.dt.bfloat16
    FP32 = mybir.dt.float32
    I16 = mybir.dt.int16
    I32 = mybir.dt.int32
    EQ = mybir.AluOpType.is_equal
    GE = mybir.AluOpType.is_ge
    MULT = mybir.AluOpType.mult
    ADD = mybir.AluOpType.add
    AF = mybir.ActivationFunctionType

    W_IN = 512

    # trace_sim=True makes the tile scheduler take a deterministic path that
    # reliably lands the fast (~1.18 ms) schedule; without it, scheduling is
    # nondeterministic across processes (measured 1.17-1.39 ms spread).
    with tile.TileContext(nc, trace_sim=True) as tc:
        with (
            tc.tile_pool(name="persist", bufs=1) as persist,
            tc.tile_pool(name="stage", bufs=2) as stage,
            tc.tile_pool(name="lhsp", bufs=POOL_LHS) as lhspool,
            tc.tile_pool(name="lhsd", bufs=POOL_LHSD) as lhsdpool,
            tc.tile_pool(name="rhsp", bufs=POOL_RHS) as rhspool,
            tc.tile_pool(name="psum", bufs=1, space="PSUM") as psum_pool,
        ):
            iota128 = persist.tile([P, 128], I16, tag="iota128")
            nc.gpsimd.iota(iota128[:, :], pattern=[[1, 128]], base=0, channel_multiplier=0)
            # interleaved iota: iota_int2[p, 2n+j] = n
            iota_int2 = persist.tile([P, 256], I16, tag="iota_int2")
            nc.gpsimd.iota(iota_int2[:, :], pattern=[[1, 128], [0, 2]], base=0, channel_multiplier=0)
            iota_int4 = persist.tile([P, 512], I16, tag="iota_int4")
            nc.gpsimd.iota(iota_int4[:, :], pattern=[[1, 128], [0, 4]], base=0, channel_multiplier=0)
            # scatter offsets 128*(c mod SB), periodic; W_IN % SB == 0
            off16 = persist.tile([P, W_IN], I16, tag="off16")
            nc.gpsimd.iota(off16[:, :], pattern=[[0, W_IN // SB], [128, SB]], base=0, channel_multiplier=0)

            imodf = persist.tile([P, w], FP32, tag="imodf")    # ACT bias (fp32)
            imod16 = persist.tile([P, w], I16, tag="imod16")   # TT compare (int16)
            tmodf = persist.tile([P, w], FP32, tag="tmodf")    # DVE lhs scalar1
            ampf = persist.tile([P, w], FP32, tag="ampf")      # DVE lhs scalar2
            idx_flat = persist.tile([P, w], I16, tag="idxf")   # tm + 128*(c%SB)
            amp_flat = persist.tile([P, w], BF16, tag="ampb")  # scatter data

            for s in range(0, w, W_IN):
                ws = min(W_IN, w - s)
                st = stage.tile([P, W_IN], I32, tag="st_t")
                nc.sync.dma_start(out=st[:, :ws], in_=tgt[:, s : s + ws])
                si = stage.tile([P, W_IN], I32, tag="st_i")
                nc.sync.dma_start(out=si[:, :ws], in_=inp[:, s : s + ws])

                t7 = stage.tile([P, W_IN], FP32, tag="t7")
                nc.vector.tensor_scalar(out=t7[:, :ws], in0=st[:, :ws], scalar1=127.5, scalar2=None, op0=GE)
                i7 = stage.tile([P, W_IN], FP32, tag="i7")
                nc.vector.tensor_scalar(out=i7[:, :ws], in0=si[:, :ws], scalar1=127.5, scalar2=None, op0=GE)

                nc.vector.scalar_tensor_tensor(out=imodf[:, s : s + ws], in0=i7[:, :ws], scalar=-128.0, in1=si[:, :ws], op0=MULT, op1=ADD)
                nc.vector.scalar_tensor_tensor(out=tmodf[:, s : s + ws], in0=t7[:, :ws], scalar=-128.0, in1=st[:, :ws], op0=MULT, op1=ADD)
                nc.vector.tensor_copy(out=imod16[:, s : s + ws], in_=imodf[:, s : s + ws])

                # amp = (1 + 4095*t7) * (1 + 63*i7); in-place on t7/i7
                nc.vector.tensor_scalar(out=t7[:, :ws], in0=t7[:, :ws], scalar1=4095.0, scalar2=1.0, op0=MULT, op1=ADD)
                nc.vector.tensor_scalar(out=i7[:, :ws], in0=i7[:, :ws], scalar1=63.0, scalar2=1.0, op0=MULT, op1=ADD)
                nc.vector.tensor_tensor(out=ampf[:, s : s + ws], in0=t7[:, :ws], in1=i7[:, :ws], op=MULT)
                nc.vector.tensor_copy(out=amp_flat[:, s : s + ws], in_=ampf[:, s : s + ws])

                # idx = tmod + 128*(c mod SB)
                tm16 = stage.tile([P, W_IN], I16, tag="tm16")
                nc.vector.tensor_copy(out=tm16[:, :ws], in_=tmodf[:, s : s + ws])
                nc.vector.tensor_tensor(out=idx_flat[:, s : s + ws], in0=tm16[:, :ws], in1=off16[:, :ws], op=ADD)

            psum = psum_pool.tile([P, 128], FP32, tag="psum")

            n_batches = w // SB
            for b in range(n_batches):
                c0 = b * SB
                in_super = b % (SUPER // SB)
                if in_super not in DVE_LHS_BATCHES and in_super not in SPLIT_BATCHES:
                    lhsb = lhspool.tile([P, SB * 128], BF16, tag="lhsb")
                    nc.gpsimd.local_scatter(
                        out_ap=lhsb[:, :],
                        data_ap=amp_flat[:, c0 : c0 + SB],
                        idxs_ap=idx_flat[:, c0 : c0 + SB],
                        channels=P, num_elems=SB * 128, num_idxs=SB,
                    )
                    lhs_of = lambda j, _t=lhsb: _t[:, 128 * j : 128 * (j + 1)]
                elif in_super in SPLIT_BATCHES:
                    # chunks c0..c0+3 via a half-width B=4 scatter (their idx
                    # offsets 128*(c mod SB) lie in [0,512)); chunks c0+4..c0+7
                    # via DVE tensor_scalar
                    half = SB // 2
                    lhsh = lhspool.tile([P, half * 128], BF16, tag="lhsh")
                    nc.gpsimd.local_scatter(
                        out_ap=lhsh[:, :],
                        data_ap=amp_flat[:, c0 : c0 + half],
                        idxs_ap=idx_flat[:, c0 : c0 + half],
                        channels=P, num_elems=half * 128, num_idxs=half,
                    )
                    dve_tiles = []
                    for j in range(half, SB):
                        c = c0 + j
                        lt = lhsdpool.tile([P, 128], BF16, tag="lhsd")
                        nc.vector.tensor_scalar(
                            out=lt[:, :], in0=iota128[:, :],
                            scalar1=tmodf[:, c : c + 1], scalar2=ampf[:, c : c + 1],
                            op0=EQ, op1=MULT,
                        )
                        dve_tiles.append(lt)
                    lhs_of = (
                        lambda j, _l=dve_tiles, _t=lhsh, _h=half:
                        _t[:, 128 * j : 128 * (j + 1)] if j < _h else _l[j - _h][:, :]
                    )
                else:
                    dve_tiles = []
                    for j in range(SB):
                        c = c0 + j
                        lt = lhsdpool.tile([P, 128], BF16, tag="lhsd")
                        nc.vector.tensor_scalar(
                            out=lt[:, :], in0=iota128[:, :],
                            scalar1=tmodf[:, c : c + 1], scalar2=ampf[:, c : c + 1],
                            op0=EQ, op1=MULT,
                        )
                        dve_tiles.append(lt)
                    lhs_of = lambda j, _l=dve_tiles: _l[j][:, :]

                if RHS_QUAD:
                    for q in range(SB // 4):
                        c = c0 + 4 * q
                        quad_idx = (c % SUPER) // 4
                        if quad_idx in ACT_QUADS:
                            rhs_aps = []
                            for j in range(4):
                                sq = rhspool.tile([P, 128], FP32, tag="sq")
                                nc.scalar.activation(
                                    out=sq[:, :], in_=iota128[:, :], func=AF.Square,
                                    bias=imodf[:, c + j : c + j + 1], scale=-1.0,
                                )
                                rt = rhspool.tile([P, 128], BF16, tag="rhs_a")
                                nc.scalar.activation(
                                    out=rt[:, :], in_=sq[:, :], func=AF.Relu,
                                    bias=1.0, scale=-1.0,
                                )
                                rhs_aps.append(rt[:, :])
                        else:
                            prt = rhspool.tile([P, 512], BF16, tag="rhs_q")
                            in1 = bass.AP(imod16.tensor, c, [[w, P], [0, 128], [1, 4]])
                            nc.vector.tensor_tensor(out=prt[:, :], in0=iota_int4[:, :], in1=in1, op=EQ)
                            rhs_aps = [bass.AP(prt.tensor, j, [[512, P], [4, 128]]) for j in range(4)]
                        for j in range(4):
                            cc = c + j
                            nc.tensor.matmul(
                                psum[:, :], lhs_of(4 * q + j), rhs_aps[j],
                                start=(cc == 0), stop=(cc == w - 1),
                            )
                    continue
                # rhs pairs within this batch: chunks (c0+2q, c0+2q+1)
                for q in range(SB // 2):
                    c = c0 + 2 * q
                    pair_idx = (c % SUPER) // 2
                    if pair_idx in ACT_PAIRS:
                        rhs_list = []
                        for j in range(2):
                            sq = rhspool.tile([P, 128], FP32, tag="sq")
                            nc.scalar.activation(
                                out=sq[:, :], in_=iota128[:, :], func=AF.Square,
                                bias=imodf[:, c + j : c + j + 1], scale=-1.0,
                            )
                            rt = rhspool.tile([P, 128], BF16, tag="rhs_a")
                            nc.scalar.activation(
                                out=rt[:, :], in_=sq[:, :], func=AF.Relu,
                                bias=1.0, scale=-1.0,
                            )
                            rhs_list.append(rt)
                        rhs_aps = [rhs_list[0][:, :], rhs_list[1][:, :]]
                    else:
                        prt = rhspool.tile([P, 256], BF16, tag="rhs_p")
                        in1 = bass.AP(imod16.tensor, c, [[w, P], [0, 128], [1, 2]])
                        nc.vector.tensor_tensor(out=prt[:, :], in0=iota_int2[:, :], in1=in1, op=EQ)
                        rhs_aps = [
                            bass.AP(prt.tensor, 0, [[256, P], [2, 128]]),
                            bass.AP(prt.tensor, 1, [[256, P], [2, 128]]),
                        ]
                    for j in range(2):
                        cc = c + j
                        nc.tensor.matmul(
                            psum[:, :], lhs_of(2 * q + j), rhs_aps[j],
                            start=(cc == 0), stop=(cc == w - 1),
                        )

            out_sb = persist.tile([P, 128], FP32, tag="out_sb")
            nc.vector.tensor_copy(out=out_sb[:, :], in_=psum[:, :])
            nc.sync.dma_start(out=hist[:, :], in_=out_sb[:, :])

    nc.compile()
    return nc


def _get_program(w=W, kind="v3"):
    key = (kind, w)
    if key not in _compiled:
        _compiled[key] = _build_program_v3(w)
    return _compiled[key]


def _decode_fields(h):
    v = np.rint(h).astype(np.int64)
    c00 = v & 63
    c01 = (v >> 6) & 63
    c10 = (v >> 12) & 63
    c11 = v >> 18
    inter = np.zeros((C, C), dtype=np.int64)
    inter[0:128, 0:128] = c00
    inter[0:128, 128:256] = c01
    inter[128:256, 0:128] = c10
    inter[128:256, 128:256] = c11
    return inter, (c00, c01, c10, c11)


def _histogram_device(input_np, target_np, w=W, trace=False):
    from concourse.bass_utils import run_bass_kernel_spmd

    n = NCORES * P * w
    inp = np.ascontiguousarray(input_np[:n].reshape(NCORES, P, w).astype(np.int32))
    tgt = np.ascontiguousarray(target_np[:n].reshape(NCORES, P, w).astype(np.int32))
    in_maps = [{"inp": inp[c], "tgt": tgt[c]} for c in range(NCORES)]

    nc = _get_program(w, "v3")
    try:
        res = run_bass_kernel_spmd(nc, in_maps, core_ids=list(range(NCORES)), trace=trace)
    except Exception:
        res = run_bass_kernel_spmd(nc, in_maps, core_ids=list(range(NCORES)), trace=trace)

    inter = np.zeros((C, C), dtype=np.int64)
    ok = True
    for cid in range(NCORES):
        h = res.results[cid]["hist"].astype(np.float64)
        ci, fields = _decode_fields(h)
        inter += ci
        if any(f.max() >= 63 for f in fields) or ci.sum() != P * w:
            ok = False
    if not ok:
        inter = np.zeros((C, C), dtype=np.int64)
        np.add.at(inter, (target_np[:n].reshape(-1), input_np[:n].reshape(-1)), 1)
        return inter.astype(np.float64), res
    return inter.astype(np.float64), res


def _finalize(inter64):
    inter = inter64.astype(np.float32)
    cnt_gt = inter.sum(axis=1, dtype=np.float32)
    cnt_pr = inter.sum(axis=0, dtype=np.float32)
    union = cnt_gt[:, None] + cnt_pr[None, :] - inter
    with np.errstate(divide="ignore", invalid="ignore"):
        iou = np.where(union > 0, inter / np.maximum(union, np.float32(1.0)), np.float32(0.0)).astype(np.float32)
    TP = (iou >= np.float32(IOU_TH)).astype(np.float32).sum(axis=1)
    FP = ((iou > 0) & (iou < np.float32(IOU_TH))).astype(np.float32).sum(axis=1)
    present = cnt_gt > 0
    precision = np.where(present, TP / np.maximum(TP + FP, np.float32(1.0)), np.float32(0.0)).astype(np.float32)
    n_gt = max(np.float32(present.astype(np.float32).sum()), np.float32(1.0))
    return np.float32(precision.sum(dtype=np.float32) / n_gt)


def kernel(input, target):
    input = np.asarray(input)
    target = np.asarray(target)
    inter, _ = _histogram_device(input, target)
    return np.array(_finalize(inter), dtype=np.float32)


if __name__ == "__main__":
    rng = np.random.default_rng(0)
    inp = rng.integers(0, C, size=N_TOTAL, dtype=np.int32)
    tgt = rng.integers(0, C, size=N_TOTAL, dtype=np.int32)
    out = kernel(input=inp, target=tgt)
    print("kernel output:", out)


# revision 6
# speedup vs baseline: 1.2525x; 1.0022x over previous
"""AveragePrecision kernel v3 for Trainium2, 8 NeuronCores.

Histogram via 4-amplitude-packed one-hot matmul (psum [128,128] fp32):
  per point k: amp_k * [t_k mod 128 == m] * [i_k mod 128 == n],
  amp_k = 4096^(t_k>=128) * 64^(i_k>=128) in {1,64,4096,262144}.
  Six-bit fields, exact in fp32 while every per-core field count < 64
  (expected ~16 for the graded input; host-side validity check with
  numpy fallback).

Builder mix per 64-chunk superblock (measured steady-state costs):
  - lhs (stationary, t-one-hot * amp):
      batches 0-6: GPSIMD batched local_scatter, 8 chunks/call (~138 ns/chunk)
      batch 7:     DVE tensor_scalar(is_equal, mult) per chunk (~228 ns)
  - rhs (moving, plain i-one-hot):
      26 of 32 pairs: one batched TT is_equal per 2 chunks against an
        interleaved iota (iota_int2[p,2n+j]=n, in1 minor-dim [1,2] keeps
        the DVE 2x perf mode, ~104 ns/chunk); the matmul consumes the
        pair tile through a stride-2 AP (4-byte step = full PE speed).
      6 of 32 pairs: ACT square+relu one-hot, contiguous (~659 ns/chunk)
  - PE: one ldweights+matmul(N=128) per chunk (~65 ns).
"""

import sys
import types

sys.path.insert(0, "/opt/trn_rl_repo")

if "antenv.axon_hooks" not in sys.modules:
    _hooks = types.ModuleType("antenv.axon_hooks")
    _hooks._HOOK = None

    def _get_hook():
        if _hooks._HOOK is None:
            try:
                from trn_agent_boot.trn_boot import _ntff_profile_via_ctypes

                _hooks._HOOK = _ntff_profile_via_ctypes("/opt/axon/libaxon_pjrt.so")
            except Exception:
                _hooks._HOOK = None
        return _hooks._HOOK

    def _set_hook(h):
        _hooks._HOOK = h

    _hooks.get_axon_ntff_profile_hook = _get_hook
    _hooks.set_axon_ntff_profile_hook = _set_hook
    sys.modules["antenv.axon_hooks"] = _hooks

import numpy as np

N_TOTAL = 8_388_608
C = 256
IOU_TH = 0.5
NCORES = 8
N_PER_CORE = N_TOTAL // NCORES
P = 128
W = N_PER_CORE // P                     # 8192 chunks per core

_compiled = {}

SB = 8            # gpsimd local_scatter batch (chunks per call)
SUPER = 64        # superblock: 8 batches
DVE_LHS_BATCHES = {3, 7}  # batches (of SUPER//SB) built by DVE instead of GPSIMD
ACT_PAIRS = {2, 7, 12, 17, 22, 27}  # pairs (of SUPER//2) built by ACT
SPLIT_BATCHES = set()  # batches half-split: B=4 scatter + 4 DVE chunks
RHS_QUAD = True  # rhs via B=4 interleaved TT (stride-4 matmul) instead of pairs
ACT_QUADS = {2, 7, 12}  # quad indices (of 16 per superblock) built by ACT when RHS_QUAD
POOL_LHS = 6
POOL_LHSD = 12
POOL_RHS = 11
PREPROC_GPS = False  # route preproc dtype-cast copies to GPSIMD instead of DVE
QUAD2 = False  # quad tile as two pair-interleaved halves: stride-2 matmul reads


def _build_program_v3(w=W):
    import concourse.bass as bass
    import concourse.mybir as mybir
    import concourse.tile as tile
    from concourse import bacc

    nc = bacc.Bacc("TRN2", target_bir_lowering=False, debug=False, num_devices=NCORES)

    inp = nc.dram_tensor("inp", [P, w], mybir.dt.int32, kind="ExternalInput").ap()
    tgt = nc.dram_tensor("tgt", [P, w], mybir.dt.int32, kind="ExternalInput").ap()
    hist = nc.dram_tensor("hist", [P, 128], mybir.dt.float32, kind="ExternalOutput").ap()

    BF16 = mybir.dt.bfloat16
    FP32 = mybir.dt.float32
    I16 = mybir.dt.int16
    I32 = mybir.dt.int32
    EQ = mybir.AluOpType.is_equal
    GE = mybir.AluOpType.is_ge
    MULT = mybir.AluOpType.mult
    ADD = mybir.AluOpType.add
    AF = mybir.ActivationFunctionType

    W_IN = 512

    # trace_sim=True makes the tile scheduler take a deterministic path that
    # reliably lands the fast (~1.18 ms) schedule; without it, scheduling is
    # nondeterministic across processes (measured 1.17-1.39 ms spread).
    with tile.TileContext(nc, trace_sim=True) as tc:
        with (
            tc.tile_pool(name="persist", bufs=1) as persist,
            tc.tile_pool(name="stage", bufs=2) as stage,
            tc.tile_pool(name="lhsp", bufs=POOL_LHS) as lhspool,
            tc.tile_pool(name="lhsd", bufs=POOL_LHSD) as lhsdpool,
            tc.tile_pool(name="rhsp", bufs=POOL_RHS) as rhspool,
            tc.tile_pool(name="psum", bufs=1, space="PSUM") as psum_pool,
        ):
            iota128 = persist.tile([P, 128], I16, tag="iota128")
            nc.gpsimd.iota(iota128[:, :], pattern=[[1, 128]], base=0, channel_multiplier=0)
            # interleaved iota: iota_int2[p, 2n+j] = n
            iota_int2 = persist.tile([P, 256], I16, tag="iota_int2")
            nc.gpsimd.iota(iota_int2[:, :], pattern=[[1, 128], [0, 2]], base=0, channel_multiplier=0)
            iota_int4 = persist.tile([P, 512], I16, tag="iota_int4")
            nc.gpsimd.iota(iota_int4[:, :], pattern=[[1, 128], [0, 4]], base=0, channel_multiplier=0)
            # two pair-interleaved halves: value r at position h*256 + 2r + j
            iota_2x2 = persist.tile([P, 512], I16, tag="iota_2x2")
            nc.gpsimd.iota(iota_2x2[:, :], pattern=[[0, 2], [1, 128], [0, 2]], base=0, channel_multiplier=0)
            # scatter offsets 128*(c mod SB), periodic; W_IN % SB == 0
            off16 = persist.tile([P, W_IN], I16, tag="off16")
            nc.gpsimd.iota(off16[:, :], pattern=[[0, W_IN // SB], [128, SB]], base=0, channel_multiplier=0)

            imodf = persist.tile([P, w], FP32, tag="imodf")    # ACT bias (fp32)
            imod16 = persist.tile([P, w], I16, tag="imod16")   # TT compare (int16)
            tmodf = persist.tile([P, w], FP32, tag="tmodf")    # DVE lhs scalar1
            ampf = persist.tile([P, w], FP32, tag="ampf")      # DVE lhs scalar2
            idx_flat = persist.tile([P, w], I16, tag="idxf")   # tm + 128*(c%SB)
            amp_flat = persist.tile([P, w], BF16, tag="ampb")  # scatter data

            for s in range(0, w, W_IN):
                ws = min(W_IN, w - s)
                st = stage.tile([P, W_IN], I32, tag="st_t")
                nc.sync.dma_start(out=st[:, :ws], in_=tgt[:, s : s + ws])
                si = stage.tile([P, W_IN], I32, tag="st_i")
                nc.sync.dma_start(out=si[:, :ws], in_=inp[:, s : s + ws])

                t7 = stage.tile([P, W_IN], FP32, tag="t7")
                nc.vector.tensor_scalar(out=t7[:, :ws], in0=st[:, :ws], scalar1=127.5, scalar2=None, op0=GE)
                i7 = stage.tile([P, W_IN], FP32, tag="i7")
                nc.vector.tensor_scalar(out=i7[:, :ws], in0=si[:, :ws], scalar1=127.5, scalar2=None, op0=GE)

                nc.vector.scalar_tensor_tensor(out=imodf[:, s : s + ws], in0=i7[:, :ws], scalar=-128.0, in1=si[:, :ws], op0=MULT, op1=ADD)
                nc.vector.scalar_tensor_tensor(out=tmodf[:, s : s + ws], in0=t7[:, :ws], scalar=-128.0, in1=st[:, :ws], op0=MULT, op1=ADD)
                _cp = nc.gpsimd.tensor_copy if PREPROC_GPS else nc.vector.tensor_copy
                _cp(out=imod16[:, s : s + ws], in_=imodf[:, s : s + ws])

                # amp = (1 + 4095*t7) * (1 + 63*i7); in-place on t7/i7
                nc.vector.tensor_scalar(out=t7[:, :ws], in0=t7[:, :ws], scalar1=4095.0, scalar2=1.0, op0=MULT, op1=ADD)
                nc.vector.tensor_scalar(out=i7[:, :ws], in0=i7[:, :ws], scalar1=63.0, scalar2=1.0, op0=MULT, op1=ADD)
                nc.vector.tensor_tensor(out=ampf[:, s : s + ws], in0=t7[:, :ws], in1=i7[:, :ws], op=MULT)
                _cp(out=amp_flat[:, s : s + ws], in_=ampf[:, s : s + ws])

                # idx = tmod + 128*(c mod SB)
                tm16 = stage.tile([P, W_IN], I16, tag="tm16")
                _cp(out=tm16[:, :ws], in_=tmodf[:, s : s + ws])
                nc.vector.tensor_tensor(out=idx_flat[:, s : s + ws], in0=tm16[:, :ws], in1=off16[:, :ws], op=ADD)

            psum = psum_pool.tile([P, 128], FP32, tag="psum")

            n_batches = w // SB
            for b in range(n_batches):
                c0 = b * SB
                in_super = b % (SUPER // SB)
                if in_super not in DVE_LHS_BATCHES and in_super not in SPLIT_BATCHES:
                    lhsb = lhspool.tile([P, SB * 128], BF16, tag="lhsb")
                    nc.gpsimd.local_scatter(
                        out_ap=lhsb[:, :],
                        data_ap=amp_flat[:, c0 : c0 + SB],
                        idxs_ap=idx_flat[:, c0 : c0 + SB],
                        channels=P, num_elems=SB * 128, num_idxs=SB,
                    )
                    lhs_of = lambda j, _t=lhsb: _t[:, 128 * j : 128 * (j + 1)]
                elif in_super in SPLIT_BATCHES:
                    # chunks c0..c0+3 via a half-width B=4 scatter (their idx
                    # offsets 128*(c mod SB) lie in [0,512)); chunks c0+4..c0+7
                    # via DVE tensor_scalar
                    half = SB // 2
                    lhsh = lhspool.tile([P, half * 128], BF16, tag="lhsh")
                    nc.gpsimd.local_scatter(
                        out_ap=lhsh[:, :],
                        data_ap=amp_flat[:, c0 : c0 + half],
                        idxs_ap=idx_flat[:, c0 : c0 + half],
                        channels=P, num_elems=half * 128, num_idxs=half,
                    )
                    dve_tiles = []
                    for j in range(half, SB):
                        c = c0 + j
                        lt = lhsdpool.tile([P, 128], BF16, tag="lhsd")
                        nc.vector.tensor_scalar(
                            out=lt[:, :], in0=iota128[:, :],
                            scalar1=tmodf[:, c : c + 1], scalar2=ampf[:, c : c + 1],
                            op0=EQ, op1=MULT,
                        )
                        dve_tiles.append(lt)
                    lhs_of = (
                        lambda j, _l=dve_tiles, _t=lhsh, _h=half:
                        _t[:, 128 * j : 128 * (j + 1)] if j < _h else _l[j - _h][:, :]
                    )
                else:
                    dve_tiles = []
                    for j in range(SB):
                        c = c0 + j
                        lt = lhsdpool.tile([P, 128], BF16, tag="lhsd")
                        nc.vector.tensor_scalar(
                            out=lt[:, :], in0=iota128[:, :],
                            scalar1=tmodf[:, c : c + 1], scalar2=ampf[:, c : c + 1],
                            op0=EQ, op1=MULT,
                        )
                        dve_tiles.append(lt)
                    lhs_of = lambda j, _l=dve_tiles: _l[j][:, :]

                if RHS_QUAD:
                    for q in range(SB // 4):
                        c = c0 + 4 * q
                        quad_idx = (c % SUPER) // 4
                        if quad_idx in ACT_QUADS:
                            rhs_aps = []
                            for j in range(4):
                                sq = rhspool.tile([P, 128], FP32, tag="sq")
                                nc.scalar.activation(
                                    out=sq[:, :], in_=iota128[:, :], func=AF.Square,
                                    bias=imodf[:, c + j : c + j + 1], scale=-1.0,
                                )
                                rt = rhspool.tile([P, 128], BF16, tag="rhs_a")
                                nc.scalar.activation(
                                    out=rt[:, :], in_=sq[:, :], func=AF.Relu,
                                    bias=1.0, scale=-1.0,
                                )
                                rhs_aps.append(rt[:, :])
                        elif QUAD2:
                            prt = rhspool.tile([P, 512], BF16, tag="rhs_q")
                            in1 = bass.AP(imod16.tensor, c, [[w, P], [2, 2], [0, 128], [1, 2]])
                            nc.vector.tensor_tensor(out=prt[:, :], in0=iota_2x2[:, :], in1=in1, op=EQ)
                            rhs_aps = [
                                bass.AP(prt.tensor, (j // 2) * 256 + (j % 2), [[512, P], [2, 128]])
                                for j in range(4)
                            ]
                        else:
                            prt = rhspool.tile([P, 512], BF16, tag="rhs_q")
                            in1 = bass.AP(imod16.tensor, c, [[w, P], [0, 128], [1, 4]])
                            nc.vector.tensor_tensor(out=prt[:, :], in0=iota_int4[:, :], in1=in1, op=EQ)
                            rhs_aps = [bass.AP(prt.tensor, j, [[512, P], [4, 128]]) for j in range(4)]
                        for j in range(4):
                            cc = c + j
                            nc.tensor.matmul(
                                psum[:, :], lhs_of(4 * q + j), rhs_aps[j],
                                start=(cc == 0), stop=(cc == w - 1),
                            )
                    continue
                # rhs pairs within this batch: chunks (c0+2q, c0+2q+1)
                for q in range(SB // 2):
                    c = c0 + 2 * q
                    pair_idx = (c % SUPER) // 2
                    if pair_idx in ACT_PAIRS:
                        rhs_list = []
                        for j in range(2):
                            sq = rhspool.tile([P, 128], FP32, tag="sq")
                            nc.scalar.activation(
                                out=sq[:, :], in_=iota128[:, :], func=AF.Square,
                                bias=imodf[:, c + j : c + j + 1], scale=-1.0,
                            )
                            rt = rhspool.tile([P, 128], BF16, tag="rhs_a")
                            nc.scalar.activation(
                                out=rt[:, :], in_=sq[:, :], func=AF.Relu,
                                bias=1.0, scale=-1.0,
                            )
                            rhs_list.append(rt)
                        rhs_aps = [rhs_list[0][:, :], rhs_list[1][:, :]]
                    else:
                        prt = rhspool.tile([P, 256], BF16, tag="rhs_p")
                        in1 = bass.AP(imod16.tensor, c, [[w, P], [0, 128], [1, 2]])
                        nc.vector.tensor_tensor(out=prt[:, :], in0=iota_int2[:, :], in1=in1, op=EQ)
                        rhs_aps = [
                            bass.AP(prt.tensor, 0, [[256, P], [2, 128]]),
                            bass.AP(prt.tensor, 1, [[256, P], [2, 128]]),
                        ]
                    for j in range(2):
                        cc = c + j
                        nc.tensor.matmul(
                            psum[:, :], lhs_of(2 * q + j), rhs_aps[j],
                            start=(cc == 0), stop=(cc == w - 1),
                        )

            out_sb = persist.tile([P, 128], FP32, tag="out_sb")
            nc.vector.tensor_copy(out=out_sb[:, :], in_=psum[:, :])
            nc.sync.dma_start(out=hist[:, :], in_=out_sb[:, :])

    nc.compile()
    return nc


def _get_program(w=W, kind="v3"):
    key = (kind, w)
    if key not in _compiled:
        _compiled[key] = _build_program_v3(w)
    return _compiled[key]


def _decode_fields(h):
    v = np.rint(h).astype(np.int64)
    c00 = v & 63
    c01 = (v >> 6) & 63
    c10 = (v >> 12) & 63
    c11 = v >> 18
    inter = np.zeros((C, C), dtype=np.int64)
    inter[0:128, 0:128] = c00
    inter[0:128, 128:256] = c01
    inter[128:256, 0:128] = c10
    inter[128:256, 128:256] = c11
    return inter, (c00, c01, c10, c11)


def _histogram_device(input_np, target_np, w=W, trace=False):
    from concourse.bass_utils import run_bass_kernel_spmd

    n = NCORES * P * w
    inp = np.ascontiguousarray(input_np[:n].reshape(NCORES, P, w).astype(np.int32))
    tgt = np.ascontiguousarray(target_np[:n].reshape(NCORES, P, w).astype(np.int32))
    in_maps = [{"inp": inp[c], "tgt": tgt[c]} for c in range(NCORES)]

    nc = _get_program(w, "v3")
    try:
        res = run_bass_kernel_spmd(nc, in_maps, core_ids=list(range(NCORES)), trace=trace)
    except Exception:
        res = run_bass_kernel_spmd(nc, in_maps, core_ids=list(range(NCORES)), trace=trace)

    inter = np.zeros((C, C), dtype=np.int64)
    ok = True
    for cid in range(NCORES):
        h = res.results[cid]["hist"].astype(np.float64)
        ci, fields = _decode_fields(h)
        inter += ci
        if any(f.max() >= 63 for f in fields) or ci.sum() != P * w:
            ok = False
    if not ok:
        inter = np.zeros((C, C), dtype=np.int64)
        np.add.at(inter, (target_np[:n].reshape(-1), input_np[:n].reshape(-1)), 1)
        return inter.astype(np.float64), res
    return inter.astype(np.float64), res


def _finalize(inter64):
    inter = inter64.astype(np.float32)
    cnt_gt = inter.sum(axis=1, dtype=np.float32)
    cnt_pr = inter.sum(axis=0, dtype=np.float32)
    union = cnt_gt[:, None] + cnt_pr[None, :] - inter
    with np.errstate(divide="ignore", invalid="ignore"):
        iou = np.where(union > 0, inter / np.maximum(union, np.float32(1.0)), np.float32(0.0)).astype(np.float32)
    TP = (iou >= np.float32(IOU_TH)).astype(np.float32).sum(axis=1)
    FP = ((iou > 0) & (iou < np.float32(IOU_TH))).astype(np.float32).sum(axis=1)
    present = cnt_gt > 0
    precision = np.where(present, TP / np.maximum(TP + FP, np.float32(1.0)), np.float32(0.0)).astype(np.float32)
    n_gt = max(np.float32(present.astype(np.float32).sum()), np.float32(1.0))
    return np.float32(precision.sum(dtype=np.float32) / n_gt)


def kernel(input, target):
    input = np.asarray(input)
    target = np.asarray(target)
    inter, _ = _histogram_device(input, target)
    return np.array(_finalize(inter), dtype=np.float32)


if __name__ == "__main__":
    rng = np.random.default_rng(0)
    inp = rng.integers(0, C, size=N_TOTAL, dtype=np.int32)
    tgt = rng.integers(0, C, size=N_TOTAL, dtype=np.int32)
    out = kernel(input=inp, target=tgt)
    print("kernel output:", out)
